# revision 31
# baseline (speedup 1.0000x reference)
"""DLRM DotInteractionArch kernel for 8x Trainium2 NeuronCores.

Problem: B=16384, 26 sparse embeddings + 1 dense feature, D=128.
  combined[b] = concat(dense[b], emb[b])           # [27, 128]
  G[b] = combined[b] @ combined[b].T               # [27, 27]
  out[b] = concat(dense[b], triu(G[b], k=1).flat)  # [479]

Strategy (pure data parallel, 2048 samples/core):
  - Load 4-sample groups as [108 partitions, 128d] bf16 (cast on DMA),
    partition p = 4f+s so each per-s load DMA touches stride-4 partitions
    (spreads across all 16 SBUF DMA ports).
  - PE transpose via identity matmul -> CT [128d, 108] in PSUM.
  - DVE cast-copy CT to SBUF, restriding per-sample feature columns into
    32-col strips.
  - Per sample: PE matmul CT_s.T @ CT_s -> G_s [27,27] into PSUM partition
    strip 32s (tile_position col tiling); 16 groups share one PSUM bank.
  - DVE copies the G bank to SBUF staging; 26 strided DMAs/round write the
    strict-upper-triangle runs into the packed output rows in HBM, split
    across the two HWDGE rings (sync/scalar) plus a gpsimd share whose
    emission is deferred one round so it does not block the next round's
    loads in the SWDGE FIFO.
  - Dense columns are a single HBM->HBM DMA.
  - Loads are issued per (s, group-half) so compute starts after half a
    round of data has landed; all staging pools are >=4 deep.

NOTE: stride-partition APs (CR[4+s::4], ST[f::32]) are invisible to the Tile
shadow-memory dependency tracker, so all RAW/WAR edges around them are wired
explicitly with add_dep_helper.

Sample mapping per core: b = r*128 + g*4 + s  (r: round, g: group 0..31, s: 0..3)
"""

import numpy as np

B_FULL = 16384
N_CORES = 8
BC = B_FULL // N_CORES  # 2048 samples per core
F = 27                  # 1 dense + 26 sparse features
D = 128
NSPARSE = 26
S = 4                   # samples per group (4*27=108 partitions)
GRP = 32                # groups per round (128 samples)
PAIRS = F * (F - 1) // 2  # 351
OUTC = D + PAIRS          # 479

_CACHE = {}


def _triu_offsets():
    # off[f] = column in the output row where G[f, f+1:] lands
    off = [D]
    for f in range(F - 1):
        off.append(off[-1] + (NSPARSE - f))
    return off


def _build_nc(bc: int = BC, debug_init: bool = False):
    from contextlib import ExitStack

    import concourse.bacc as bacc
    import concourse.tile as tile
    from concourse import mybir
    from concourse.masks import make_identity
    from concourse.tile_rust import add_dep_helper

    BF = mybir.dt.bfloat16
    F32 = mybir.dt.float32
    rounds = bc // (S * GRP)

    nc = bacc.Bacc("TRN2", target_bir_lowering=False, debug=False)
    den = nc.dram_tensor("dense_output", [bc, D], F32, kind="ExternalInput")
    emb = nc.dram_tensor("embeddings", [bc, NSPARSE, D], F32, kind="ExternalInput")
    out = nc.dram_tensor("out", [bc, OUTC], F32, kind="ExternalOutput")

    off = _triu_offsets()

    # b = r*128 + g*4 + s, with g = h*16 + q
    emb_v = emb.ap().rearrange("(r g s) j d -> r s j g d", g=GRP, s=S)
    den_v = den.ap().rearrange("(r g s) d -> r s g d", g=GRP, s=S)
    out_v = out.ap().rearrange("(r h q s) c -> r s h q c", h=2, q=16, s=S)

    with tile.TileContext(nc) as tc, ExitStack() as ctx:
        const = ctx.enter_context(tc.tile_pool(name="const", bufs=1))
        crp = ctx.enter_context(tc.tile_pool(name="cr", bufs=4))
        ctsp = ctx.enter_context(tc.tile_pool(name="cts", bufs=6))
        stp = ctx.enter_context(tc.tile_pool(name="st", bufs=4))
        ptp = ctx.enter_context(tc.tile_pool(name="pt", bufs=4, space="PSUM"))
        psgp = ctx.enter_context(tc.tile_pool(name="psg", bufs=4, space="PSUM"))

        ident = const.tile([108, 108], BF)
        make_identity(nc, ident)

        # dense passthrough columns: one HBM->HBM DMA
        nc.scalar.dma_start(out=out.ap()[:, 0:D], in_=den.ap()[:, :])

        CR_BUFS, ST_BUFS = 4, 4
        emb_dmas = {}   # round -> [BassInstruction] (stride-partition writes)
        mma_insts = {}  # round -> [BassInstruction] (CR readers)
        st_copies = {}  # round -> [BassInstruction]
        st_dmas = {}    # round -> [BassInstruction]
        deferred_stores = []  # (round, dst_ap, src_ap) awaiting gpsimd emission

        for r in range(rounds):
            # ---- load: combined [108 = 4f+s, GRP, D] bf16 (cast on DMA) ----
            CR = crp.tile([108, GRP, D], BF)
            for gh in range(2):  # g-halves so compute can start on half a round
                for s in range(S):
                    d_e = nc.gpsimd.dma_start(
                        out=CR[4 + s::4, 16 * gh:16 * gh + 16],
                        in_=emb_v[r][s][:, 16 * gh:16 * gh + 16],
                    )
                    emb_dmas.setdefault((r, gh), []).append(d_e)
                    # WAR: slot is reused CR_BUFS rounds later
                    for m in mma_insts.get(r - CR_BUFS, []):
                        add_dep_helper(d_e.ins, m.ins, reason="CR slot WAR")
            # dense rows f=0 land on contiguous partitions 0..3
            nc.gpsimd.dma_start(out=CR[0:S], in_=den_v[r])

            # flush deferred gpsimd stores (previous round's data)
            for (rr, dst, src) in deferred_stores:
                d = nc.gpsimd.dma_start(out=dst, in_=src)
                st_dmas.setdefault(rr, []).append(d)
                for cpy in st_copies[rr]:
                    add_dep_helper(d.ins, cpy.ins, reason="triu DMA RAW on ST")
            deferred_stores = []

            ST = stp.tile([128, 2, 16, 32], F32)
            for h in range(2):  # halves of 16 groups
                PSG = psgp.tile([128, 16, 32], F32)
                if debug_init:
                    nc.vector.memset(PSG[:], 0.0)
                for q4 in range(4):  # 4-group transpose batches
                    PT = ptp.tile([128, 4, 108], F32)
                    CTS = ctsp.tile([128, 4, S, 32], BF)
                    if debug_init:
                        nc.vector.memset(CTS[:], 0.0)
                    for j in range(4):
                        g = h * 16 + q4 * 4 + j
                        # CT = C_grp.T : [128d, 108]
                        m = nc.tensor.matmul(
                            PT[:, j],
                            CR[:, g],
                            ident[:],
                            start=True,
                            stop=True,
                        )
                        mma_insts.setdefault(r, []).append(m)
                        for d_e in emb_dmas[(r, g // 16)]:
                            add_dep_helper(m.ins, d_e.ins, reason="MM_A RAW on CR")
                    # cast fp32->bf16; PT col order is (f-major, s-minor)
                    pin = PT[:].rearrange("p j (f s) -> p j s f", s=S)
                    nc.vector.tensor_copy(CTS[:, :, :, 0:F], pin)
                    for j in range(4):
                        q16 = q4 * 4 + j
                        for s in range(S):
                            ctsl = CTS[:, j, s, 0:F]
                            nc.tensor.matmul(
                                PSG[32 * s:32 * s + F, q16, 0:F],
                                ctsl,
                                ctsl,
                                start=True,
                                stop=True,
                                tile_position=(0, 32 * s),
                            )
                cpy = nc.vector.tensor_copy(ST[:, h], PSG[:])
                st_copies.setdefault(r, []).append(cpy)
                # WAR: this copy reuses the ST slot read by round r-2's DMAs
                for d in st_dmas.get(r - ST_BUFS, []):
                    add_dep_helper(cpy.ins, d.ins, reason="ST slot WAR")

            # ---- strict upper triangle -> packed output rows ----
            # Emit this round's store DMAs; the gpsimd share is deferred one
            # round so it lands in the gpsimd FIFO after the next round's
            # loads (otherwise it blocks them).
            for f in range(F - 1):
                n = NSPARSE - f
                src = ST[f::32, :, :, f + 1:F]
                dst = out_v[r][:, :, :, off[f]:off[f] + n]
                if f >= 21:
                    deferred_stores.append((r, dst, src))
                    continue
                eng = nc.sync if f % 2 == 0 else nc.scalar
                d = eng.dma_start(out=dst, in_=src)
                st_dmas.setdefault(r, []).append(d)
                for cpy in st_copies[r]:
                    add_dep_helper(d.ins, cpy.ins, reason="triu DMA RAW on ST")

        # flush the last round's deferred stores
        for (rr, dst, src) in deferred_stores:
            d = nc.gpsimd.dma_start(out=dst, in_=src)
            for cpy in st_copies[rr]:
                add_dep_helper(d.ins, cpy.ins, reason="triu DMA RAW on ST")

    nc.finalize()
    return nc


def kernel(dense_output: np.ndarray, embeddings: np.ndarray) -> np.ndarray:
    from concourse.bass_utils import run_bass_kernel_spmd

    if "nc" not in _CACHE:
        _CACHE["nc"] = _build_nc()
    nc = _CACHE["nc"]

    dense_output = np.ascontiguousarray(np.asarray(dense_output, dtype=np.float32))
    embeddings = np.ascontiguousarray(np.asarray(embeddings, dtype=np.float32))
    in_maps = []
    for i in range(N_CORES):
        sl = slice(i * BC, (i + 1) * BC)
        in_maps.append(
            {
                "dense_output": np.ascontiguousarray(dense_output[sl]),
                "embeddings": np.ascontiguousarray(embeddings[sl]),
            }
        )
    res = run_bass_kernel_spmd(nc, in_maps, list(range(N_CORES)))
    return np.concatenate([res.results[i]["out"] for i in range(N_CORES)], axis=0)
